# revision 4
# baseline (speedup 1.0000x reference)
"""Trainium2 Bass kernel for a causal multi-head attention layer.

Model: b=2, s=2048, d_model=1024, 16 heads, head_dim=64, pad-index 0.
Sharding over 8 NeuronCores: each core owns 2 heads (128 of the 1024
attention dims) for both batches (head/tensor parallel).  After attention,
AllToAlls redistribute the per-head outputs so each core holds all 1024
attention dims for 1/8 of the sequence positions, where it runs the output
projection locally.  Output rows per core: 512 (4 chunks of 128).

Schedule: the exp of the attention scores (ACT engine, ~88us for both
batches) is the hidden critical chain, so score matmuls for BOTH batches
are emitted as early as possible and paced against the ACT engine's
throughput, with projections / PV / output-projection matmuls as filler.
The A2A runs as 4 collectives (one per half-batch) fired as soon as each
half is normalized, so only the last 128-row output-projection chunk
trails the final collective.
"""

import threading

import numpy as np

B, S, D = 2, 2048, 1024
H, HD = 16, 64
NCORES = 8
LD = D // NCORES          # 128 local attention dims (2 heads)
R = B * S                 # 4096 flattened rows
RC = R // NCORES          # 512 output rows per core
NKT = S // 128            # 16 key tiles per batch
NCH = D // 128            # 8 contraction chunks of d_model
NST = S // 512            # 4 query stripes per batch

_cache = {}
_lock = threading.Lock()


def _stripe_layout():
    """Per stripe c: list of (kt, width, q_start, offset-in-block), block len."""
    layout = []
    for c in range(NST):
        entries = []
        off = 0
        for kt in range(4 * c + 4):
            qs = max(512 * c, kt * 128)
            w = 512 * (c + 1) - qs
            entries.append((kt, w, qs, off))
            off += w
        layout.append((entries, off))
    return layout


def _build_nc():
    import concourse.mybir as mybir
    import concourse.tile as tile
    from concourse import bacc
    from contextlib import ExitStack

    f32 = mybir.dt.float32
    bf16 = mybir.dt.bfloat16
    i32 = mybir.dt.int32
    AF = mybir.ActivationFunctionType
    ALU = mybir.AluOpType

    nc = bacc.Bacc(None, target_bir_lowering=False, num_devices=NCORES)

    xT = nc.declare_dram_parameter("xT", [D, R], bf16, isOutput=False)
    wqT = nc.declare_dram_parameter("wqT", [D, LD], bf16, isOutput=False)
    wkT = nc.declare_dram_parameter("wkT", [D, LD], bf16, isOutput=False)
    wvT = nc.declare_dram_parameter("wvT", [D, LD], bf16, isOutput=False)
    woT = nc.declare_dram_parameter("woT", [D, D], bf16, isOutput=False)
    bq = nc.declare_dram_parameter("bq", [LD], f32, isOutput=False)
    bk = nc.declare_dram_parameter("bk", [LD], f32, isOutput=False)
    bv = nc.declare_dram_parameter("bv", [LD], f32, isOutput=False)
    bo = nc.declare_dram_parameter("bo", [D], f32, isOutput=False)
    ids = nc.declare_dram_parameter("ids", [128, B * NKT], i32, isOutput=False)
    out = nc.declare_dram_parameter("out", [RC, D], f32, isOutput=True)

    layout = _stripe_layout()

    with ExitStack() as ctx:
        tc = ctx.enter_context(tile.TileContext(nc))
        const = ctx.enter_context(tc.tile_pool(name="const", bufs=1))
        xcp = ctx.enter_context(tc.tile_pool(name="xcp", bufs=1))
        qkp = ctx.enter_context(tc.tile_pool(name="qkp", bufs=2))
        estp = ctx.enter_context(tc.tile_pool(name="estp", bufs=1))
        stg = ctx.enter_context(tc.tile_pool(name="stg", bufs=2))
        work = ctx.enter_context(tc.tile_pool(name="work", bufs=2))
        recp = ctx.enter_context(tc.tile_pool(name="recp", bufs=1))
        ppool = ctx.enter_context(tc.tile_pool(name="ppool", bufs=2, space="PSUM"))
        spool = ctx.enter_context(tc.tile_pool(name="spool", bufs=2, space="PSUM"))
        pvpool = ctx.enter_context(tc.tile_pool(name="pvpool", bufs=2, space="PSUM"))
        dpool = ctx.enter_context(tc.tile_pool(name="dram", bufs=4, space="DRAM"))

        # ---- constants on the GpSimd DMA queue so the x loads own Sync ----
        wqT_sb = const.tile([128, NCH, LD], bf16)
        nc.gpsimd.dma_start(wqT_sb, wqT.ap().rearrange("(c p) d -> p c d", p=128))
        wkT_sb = const.tile([128, NCH, LD], bf16)
        nc.gpsimd.dma_start(wkT_sb, wkT.ap().rearrange("(c p) d -> p c d", p=128))
        bq_col = const.tile([128, 1], f32)
        nc.gpsimd.dma_start(bq_col, bq.ap().rearrange("(p o) -> p o", o=1))
        bk_col = const.tile([128, 1], f32)
        nc.gpsimd.dma_start(bk_col, bk.ap().rearrange("(p o) -> p o", o=1))
        wvT_sb = const.tile([128, NCH, LD], bf16)
        nc.gpsimd.dma_start(wvT_sb, wvT.ap().rearrange("(c p) d -> p c d", p=128))
        bv_bc = const.tile([128, LD], f32)
        nc.gpsimd.dma_start(bv_bc, bv.ap().partition_broadcast(128))
        ids_sb = const.tile([128, B * NKT], i32)
        nc.gpsimd.dma_start(ids_sb, ids.ap())
        woT_sb = const.tile([128, NCH, D], bf16)
        nc.gpsimd.dma_start(woT_sb, woT.ap().rearrange("(c p) n -> p c n", p=128))
        bo_bc = const.tile([128, D], f32)
        nc.gpsimd.dma_start(bo_bc, bo.ap().partition_broadcast(128))

        ones64 = const.tile([1, 64], bf16)
        nc.vector.memset(ones64, 1.0)

        # x^T for batch 0, chunked by (row-block, dim-chunk) so projection
        # matmuls start as soon as the first row block lands
        xTr = xT.ap().rearrange("(c p) r -> c p r", p=128)
        xc = [xcp.tile([128, S], bf16, name=f"xc{c}", tag=f"xc{c}")
              for c in range(NCH)]
        for rb in range(4):
            rsl = slice(rb * 512, (rb + 1) * 512)
            for c in range(NCH):
                nc.sync.dma_start(xc[c][:, rsl], xTr[c][:, rsl])

        padf = const.tile([128, B * NKT], f32)
        nc.vector.tensor_copy(padf, ids_sb)
        nc.vector.tensor_scalar_min(padf, padf, 1.0)

        # diagmask2[x, h, y] = 1 if y >= x else 0 (keys on partitions)
        diagmask = const.tile([128, 128], bf16)
        nc.gpsimd.memset(diagmask, 1.0)
        nc.gpsimd.affine_select(
            out=diagmask, in_=diagmask, compare_op=ALU.is_ge, fill=0.0,
            base=0, pattern=[[1, 128]], channel_multiplier=-1,
        )
        diagmask2 = const.tile([128, 2, 128], bf16)
        nc.vector.tensor_copy(diagmask2[:, 0, :], diagmask)
        nc.vector.tensor_copy(diagmask2[:, 1, :], diagmask)

        # ---- per-batch persistent tiles ----
        qt = {}
        kt_ = {}
        vaug = {}
        stage = {}
        ests = {}
        pos = {}
        recbs = {}
        a2a_outs = {}

        EST_BUFS = [2, 2, 1, 1]

        def get_batch_tiles(b):
            if b in qt:
                return
            qt[b] = qkp.tile([128, S], bf16, name=f"qt{b}", tag="qt")
            kt_[b] = qkp.tile([128, S], bf16, name=f"kt{b}", tag="kt")
            vaug[b] = qkp.tile([128, 2, NKT, HD + 1], bf16, name=f"vaug{b}",
                               tag="vaug")
            stage[b] = stg.tile([128, S], bf16, name=f"stage{b}", tag="stage")
            ests[b] = [estp.tile([128, 2, blocklen], bf16, name=f"est{c}",
                                 tag=f"est{c}", bufs=EST_BUFS[c])
                       for c, (_, blocklen) in enumerate(layout)]

        # ---- worklets ----
        def qk(b, rb):
            get_batch_tiles(b)
            rsl = slice(rb * 512, (rb + 1) * 512)
            pqt = ppool.tile([128, 512], f32, name="pqt", tag="pp")
            pkt = ppool.tile([128, 512], f32, name="pkt", tag="pp")
            for c in range(NCH):
                st = c == 0
                sp = c == NCH - 1
                rhs = xc[c][:, rsl]
                nc.tensor.matmul(pqt, wqT_sb[:, c, :], rhs, start=st, stop=sp)
                nc.tensor.matmul(pkt, wkT_sb[:, c, :], rhs, start=st, stop=sp)
            nc.vector.tensor_scalar_add(qt[b][:, rsl], pqt, bq_col)
            nc.vector.tensor_scalar_add(kt_[b][:, rsl], pkt, bk_col)

        def vproj(b, m):
            msl = slice(m * 128, (m + 1) * 128)
            pv = ppool.tile([128, LD], f32, name="pv", tag="pp")
            for c in range(NCH):
                nc.tensor.matmul(pv, xc[c][:, msl], wvT_sb[:, c, :],
                                 start=(c == 0), stop=(c == NCH - 1))
            tv = work.tile([128, LD], f32, name="tv", tag="tv")
            nc.vector.tensor_add(tv, pv, bv_bc)
            pcol = padf[:, b * NKT + m:b * NKT + m + 1]
            for h in range(2):
                nc.vector.tensor_scalar_mul(
                    vaug[b][:, h, m, 0:HD], tv[:, h * HD:(h + 1) * HD], pcol)
                nc.vector.tensor_copy(vaug[b][:, h, m, HD:HD + 1], pcol)

        def sc(b, c, e):
            kt, w, qs, off = layout[c][0][e]
            est = ests[b][c]
            ksl = slice(kt * 128, (kt + 1) * 128)
            ps = spool.tile([128, 2, 512], f32, name="ps", tag="sp")
            nc.tensor.matmul(ps[:, 0, 0:w], kt_[b][0:64, ksl],
                             qt[b][0:64, qs:qs + w], start=True, stop=True)
            nc.tensor.matmul(ps[:, 1, 0:w], kt_[b][64:128, ksl],
                             qt[b][64:128, qs:qs + w], start=True, stop=True)
            nc.scalar.activation(est[:, :, off:off + w], ps[:, :, 0:w],
                                 AF.Exp, scale=0.125)
            if kt >= 4 * c:  # diagonal tile: causal mask
                nc.vector.tensor_mul(est[:, :, off:off + 128],
                                     est[:, :, off:off + 128], diagmask2)

        def pv(b, c):
            entries, _ = layout[c]
            est = ests[b][c]
            for h in range(2):
                po = pvpool.tile([128, 512], f32, name=f"po{h}", tag="po")
                pos[(b, c, h)] = po
                last = 4 * c + 3
                for kt, w, qs, off in entries:
                    po_off = qs - 512 * c
                    nc.tensor.matmul(po[0:HD + 1, po_off:po_off + w],
                                     vaug[b][:, h, kt, :],
                                     est[:, h, off:off + w],
                                     start=(kt == 0), stop=(kt == last))
                # start the reciprocal chain immediately (DVE) so the
                # broadcast matmul issued later never stalls the PE
                den = recp.tile([1, 512], f32, name="den", tag=f"den{h}")
                nc.vector.tensor_copy(den, po[HD:HD + 1, :])
                rec = recp.tile([1, 512], f32, name="rec", tag=f"rec{h}")
                nc.vector.reciprocal_approx_fast(rec, den)
                recb = recp.tile([1, 512], bf16, name="recb",
                                 tag=f"recb{h}", bufs=2)
                nc.vector.tensor_copy(recb, rec)
                recbs[(b, c, h)] = recb

        def div(b, c):
            # softmax division: broadcast 1/denominator to 64 partitions
            # via a tiny PE matmul into po's upper half, then multiply
            for h in range(2):
                nc.tensor.matmul(pos[(b, c, h)][64:128, :], ones64,
                                 recbs[(b, c, h)], start=True, stop=True,
                                 skip_group_check=True)
            for h in range(2):
                po = pos[(b, c, h)]
                rbc = recp.tile([HD, 512], bf16, name="rbc", tag=f"rbc{h}")
                nc.vector.tensor_copy(rbc, po[64:128, :])
                nc.vector.tensor_mul(
                    stage[b][h * HD:(h + 1) * HD, 512 * c:512 * (c + 1)],
                    po[0:HD, :], rbc)

        def a2a(b, h2):
            q0, q1 = 1024 * h2, 1024 * (h2 + 1)
            nq = (q1 - q0) // NCORES
            a2a_in = dpool.tile([NCORES * 128, nq], bf16,
                                name=f"a2ai{b}{h2}", tag="a2ai")
            nc.gpsimd.dma_start(
                a2a_in.rearrange("(j p) r -> p j r", p=128),
                stage[b][:, q0:q1].rearrange("p (j r) -> p j r", j=NCORES))
            a2a_out = dpool.tile([NCORES * 128, nq], bf16,
                                 name=f"a2ao{b}{h2}", tag="a2ao")
            nc.gpsimd.collective_compute(
                "AllToAll", ALU.bypass,
                replica_groups=[list(range(NCORES))],
                ins=[a2a_in.opt()], outs=[a2a_out.opt()])
            a2a_outs[(b, h2)] = a2a_out

        def op(b, h2):
            a2a_out = a2a_outs[(b, h2)]
            nq = 128
            a2a_sb = stg.tile([128, NCORES, nq], bf16, name=f"a2as{b}{h2}",
                              tag="a2as", bufs=3)
            nc.sync.dma_start(
                a2a_sb, a2a_out.rearrange("(j p) r -> p j r", p=128))
            r0 = (2 * b + h2) * 128
            for n in range(D // 512):
                pout = ppool.tile([128, 512], f32, name="pout", tag="pp")
                for c in range(NCH):
                    nc.tensor.matmul(
                        pout, a2a_sb[:, c, :],
                        woT_sb[:, c, n * 512:(n + 1) * 512],
                        start=(c == 0), stop=(c == NCH - 1))
                ot = work.tile([128, 512], f32, name="ot", tag="ot")
                nc.vector.tensor_add(ot, pout, bo_bc[:, n * 512:(n + 1) * 512])
                nc.sync.dma_start(
                    out.ap()[r0:r0 + 128, n * 512:(n + 1) * 512], ot)

        def xc_load(b):
            for rb in range(4):
                rsl = slice(rb * 512, (rb + 1) * 512)
                dsl = slice(b * S + rb * 512, b * S + (rb + 1) * 512)
                for c in range(NCH):
                    nc.sync.dma_start(xc[c][:, rsl], xTr[c][:, dsl])

        # ---- schedule: fillers paced against the ACT (exp) queue ----
        # sc_ready: FIFO of score entries whose qt/kt row-blocks exist.
        sc_ready = []
        act_cost = [0.0]   # us of exp work emitted
        pe_cost = [0.0]    # us of filler PE work emitted

        def enqueue_scores(b, c):
            for e in range(len(layout[c][0])):
                sc_ready.append((b, c, e))

        def drain(us=None, upto=None):
            """Emit queued score entries: ~us worth of ACT time, or all
            entries up to and including stripe (b, c) if upto is set."""
            target = act_cost[0] + (us if us is not None else 0.0)
            while sc_ready:
                b, c, e = sc_ready[0]
                if upto is not None:
                    if (b, c) > upto:
                        break
                elif act_cost[0] >= target or act_cost[0] > pe_cost[0] + 6.0:
                    break
                sc_ready.pop(0)
                sc(b, c, e)
                w = layout[c][0][e][1]
                act_cost[0] += 2 * w * 0.00109 + 0.1

        def fill(fn, us, *args):
            fn(*args)
            pe_cost[0] += us
            drain(us)

        QK_US, V_US, PV_US, OP_US = 3.8, 0.5, 0.0, 3.8

        # batch 0 projections + scores
        for rb in range(4):
            fill(qk, QK_US, 0, rb)
            enqueue_scores(0, rb)
            drain(0.0)
        for m in range(NKT):
            fill(vproj, V_US, 0, m)
        xc_load(1)

        # batch 1 projections + batch 0 PV, batch 0 A2As
        for rb in range(4):
            fill(qk, QK_US, 1, rb)
            enqueue_scores(1, rb)
            drain(0.0)
            if rb >= 2:
                c = rb - 2
                drain(upto=(0, c))
                fill(pv, 3.0, 0, c)
                fill(div, 0.5, 0, c)
        a2a(0, 0)
        for m in range(NKT):
            fill(vproj, V_US, 1, m)
            if m == 7:
                drain(upto=(0, 2))
                fill(pv, 4.5, 0, 2)
                fill(div, 0.5, 0, 2)
        drain(upto=(0, 3))
        fill(pv, 6.5, 0, 3)
        fill(div, 0.5, 0, 3)
        a2a(0, 1)

        # batch 1 PV + A2As, output projections woven in
        drain(upto=(1, 0))
        fill(pv, 1.2, 1, 0)
        fill(div, 0.5, 1, 0)
        drain(upto=(1, 1))
        fill(pv, 3.0, 1, 1)
        fill(div, 0.5, 1, 1)
        a2a(1, 0)
        drain(upto=(1, 2))
        fill(pv, 4.5, 1, 2)
        fill(div, 0.5, 1, 2)
        fill(op, OP_US, 0, 0)
        drain(upto=(1, 3))
        fill(op, OP_US, 0, 1)
        fill(pv, 6.5, 1, 3)
        fill(div, 0.5, 1, 3)
        a2a(1, 1)
        fill(op, OP_US, 1, 0)
        fill(op, OP_US, 1, 1)

        assert not sc_ready

    nc.finalize()
    return nc


def _get_nc():
    with _lock:
        if "nc" not in _cache:
            _cache["nc"] = _build_nc()
        return _cache["nc"]


def _shard_inputs(x, input_ids, Wq, bq, Wk, bk, Wv, bv, Wo, bo):
    import ml_dtypes
    bf16 = ml_dtypes.bfloat16

    x = np.asarray(x, dtype=np.float32)
    xT = np.ascontiguousarray(x.reshape(R, D).T).astype(bf16)
    woT = np.ascontiguousarray(np.asarray(Wo, dtype=np.float32).T).astype(bf16)
    bo_f = np.asarray(bo, dtype=np.float32)
    ids = np.asarray(input_ids).astype(np.int32)
    # ids_r[p, b*NKT + t] = input_ids[b, t*128 + p]
    ids_r = np.ascontiguousarray(ids.reshape(B, NKT, 128).transpose(2, 0, 1)
                                 .reshape(128, B * NKT))
    Wq = np.asarray(Wq, dtype=np.float32)
    Wk = np.asarray(Wk, dtype=np.float32)
    Wv = np.asarray(Wv, dtype=np.float32)
    bq = np.asarray(bq, dtype=np.float32)
    bk = np.asarray(bk, dtype=np.float32)
    bv = np.asarray(bv, dtype=np.float32)

    in_maps = []
    for c in range(NCORES):
        sl = slice(c * LD, (c + 1) * LD)
        in_maps.append({
            "xT": xT,
            "wqT": np.ascontiguousarray(Wq[sl].T).astype(bf16),
            "wkT": np.ascontiguousarray(Wk[sl].T).astype(bf16),
            "wvT": np.ascontiguousarray(Wv[sl].T).astype(bf16),
            "woT": woT,
            "bq": bq[sl].copy(),
            "bk": bk[sl].copy(),
            "bv": bv[sl].copy(),
            "bo": bo_f,
            "ids": ids_r,
        })
    return in_maps


def run(trace=False, **inputs):
    """Run the kernel; returns (output, BassKernelResults)."""
    from concourse.bass_utils import run_bass_kernel_spmd

    nc = _get_nc()
    in_maps = _shard_inputs(**inputs)
    res = run_bass_kernel_spmd(nc, in_maps, core_ids=list(range(NCORES)),
                               trace=trace)
    full = np.empty((B, S, D), dtype=np.float32)
    for j in range(NCORES):
        o = np.asarray(res.results[j]["out"], dtype=np.float32)
        for b in range(B):
            for h2 in range(2):
                full[b, 1024 * h2 + 128 * j:1024 * h2 + 128 * (j + 1), :] = \
                    o[(2 * b + h2) * 128:(2 * b + h2 + 1) * 128, :]
    return full, res


def kernel(**inputs) -> np.ndarray:
    full, _ = run(trace=False, **inputs)
    return full


# revision 6
# speedup vs baseline: 1.1059x; 1.1059x over previous
"""Trainium2 Bass kernel for a causal multi-head attention layer.

Model: b=2, s=2048, d_model=1024, 16 heads, head_dim=64, pad-index 0.
Sharding over 8 NeuronCores: each core owns 2 heads (128 of the 1024
attention dims) for both batches (head/tensor parallel).  After attention,
AllToAlls redistribute the per-head outputs so each core holds all 1024
attention dims for 1/8 of the sequence positions, where it runs the output
projection locally.  Output rows per core: 512 (4 chunks of 128).

Schedule: the exp of the attention scores (ACT engine, ~88us for both
batches) is the hidden critical chain, so score matmuls for BOTH batches
are emitted as early as possible and paced against the ACT engine's
throughput, with projections / PV / output-projection matmuls as filler.
The A2A runs as 4 collectives (one per half-batch) fired as soon as each
half is normalized, so only the last 128-row output-projection chunk
trails the final collective.
"""

import threading

import numpy as np

B, S, D = 2, 2048, 1024
H, HD = 16, 64
NCORES = 8
LD = D // NCORES          # 128 local attention dims (2 heads)
R = B * S                 # 4096 flattened rows
RC = R // NCORES          # 512 output rows per core
NKT = S // 128            # 16 key tiles per batch
NCH = D // 128            # 8 contraction chunks of d_model
NST = S // 512            # 4 query stripes per batch

_cache = {}
_lock = threading.Lock()


def _stripe_layout():
    """Per stripe c: list of (kt, width, q_start, offset-in-block), block len."""
    layout = []
    for c in range(NST):
        entries = []
        off = 0
        for kt in range(4 * c + 4):
            qs = max(512 * c, kt * 128)
            w = 512 * (c + 1) - qs
            entries.append((kt, w, qs, off))
            off += w
        layout.append((entries, off))
    return layout


def _build_nc():
    import concourse.mybir as mybir
    import concourse.tile as tile
    from concourse import bacc
    from contextlib import ExitStack

    f32 = mybir.dt.float32
    bf16 = mybir.dt.bfloat16
    i32 = mybir.dt.int32
    AF = mybir.ActivationFunctionType
    ALU = mybir.AluOpType

    nc = bacc.Bacc(None, target_bir_lowering=False, num_devices=NCORES)

    xT = nc.declare_dram_parameter("xT", [D, R], bf16, isOutput=False)
    wqT = nc.declare_dram_parameter("wqT", [D, LD], bf16, isOutput=False)
    wkT = nc.declare_dram_parameter("wkT", [D, LD], bf16, isOutput=False)
    wvT = nc.declare_dram_parameter("wvT", [D, LD], bf16, isOutput=False)
    woT = nc.declare_dram_parameter("woT", [D, D], bf16, isOutput=False)
    bq = nc.declare_dram_parameter("bq", [LD], f32, isOutput=False)
    bk = nc.declare_dram_parameter("bk", [LD], f32, isOutput=False)
    bv = nc.declare_dram_parameter("bv", [LD], f32, isOutput=False)
    bo = nc.declare_dram_parameter("bo", [D], f32, isOutput=False)
    ids = nc.declare_dram_parameter("ids", [128, B * NKT], i32, isOutput=False)
    out = nc.declare_dram_parameter("out", [RC, D], f32, isOutput=True)

    layout = _stripe_layout()

    with ExitStack() as ctx:
        tc = ctx.enter_context(tile.TileContext(nc))
        const = ctx.enter_context(tc.tile_pool(name="const", bufs=1))
        xcp = ctx.enter_context(tc.tile_pool(name="xcp", bufs=1))
        qkp = ctx.enter_context(tc.tile_pool(name="qkp", bufs=2))
        estp = ctx.enter_context(tc.tile_pool(name="estp", bufs=1))
        stg = ctx.enter_context(tc.tile_pool(name="stg", bufs=2))
        work = ctx.enter_context(tc.tile_pool(name="work", bufs=2))
        recp = ctx.enter_context(tc.tile_pool(name="recp", bufs=1))
        ppool = ctx.enter_context(tc.tile_pool(name="ppool", bufs=2, space="PSUM"))
        spool = ctx.enter_context(tc.tile_pool(name="spool", bufs=2, space="PSUM"))
        pvpool = ctx.enter_context(tc.tile_pool(name="pvpool", bufs=2, space="PSUM"))
        dpool = ctx.enter_context(tc.tile_pool(name="dram", bufs=4, space="DRAM"))

        # ---- dummy collective first: syncs the cores and absorbs the
        # collective-stream warmup while the compute phase runs ----
        dummy_i = dpool.tile([8, 16], bf16, name="dummy_i", tag="dummy_i")
        dummy_o = dpool.tile([8, 16], bf16, name="dummy_o", tag="dummy_o")
        nc.gpsimd.collective_compute(
            "AllToAll", ALU.bypass, replica_groups=[list(range(NCORES))],
            ins=[dummy_i.opt()], outs=[dummy_o.opt()])

        # ---- constants on the GpSimd DMA queue so the x loads own Sync ----
        wqT_sb = const.tile([128, NCH, LD], bf16)
        nc.gpsimd.dma_start(wqT_sb, wqT.ap().rearrange("(c p) d -> p c d", p=128))
        wkT_sb = const.tile([128, NCH, LD], bf16)
        nc.gpsimd.dma_start(wkT_sb, wkT.ap().rearrange("(c p) d -> p c d", p=128))
        bq_col = const.tile([128, 1], f32)
        nc.gpsimd.dma_start(bq_col, bq.ap().rearrange("(p o) -> p o", o=1))
        bk_col = const.tile([128, 1], f32)
        nc.gpsimd.dma_start(bk_col, bk.ap().rearrange("(p o) -> p o", o=1))
        wvT_sb = const.tile([128, NCH, LD], bf16)
        nc.gpsimd.dma_start(wvT_sb, wvT.ap().rearrange("(c p) d -> p c d", p=128))
        bv_bc = const.tile([128, LD], f32)
        nc.gpsimd.dma_start(bv_bc, bv.ap().partition_broadcast(128))
        ids_sb = const.tile([128, B * NKT], i32)
        nc.gpsimd.dma_start(ids_sb, ids.ap())
        woT_sb = const.tile([128, NCH, D], bf16)
        nc.gpsimd.dma_start(woT_sb, woT.ap().rearrange("(c p) n -> p c n", p=128))
        bo_bc = const.tile([128, D], f32)
        nc.gpsimd.dma_start(bo_bc, bo.ap().partition_broadcast(128))

        ones64 = const.tile([1, 64], bf16)
        nc.vector.memset(ones64, 1.0)

        # x^T for batch 0, chunked by (row-block, dim-chunk) so projection
        # matmuls start as soon as the first row block lands
        xTr = xT.ap().rearrange("(c p) r -> c p r", p=128)
        xc = [xcp.tile([128, S], bf16, name=f"xc{c}", tag=f"xc{c}")
              for c in range(NCH)]
        for rb in range(4):
            rsl = slice(rb * 512, (rb + 1) * 512)
            for c in range(NCH):
                nc.sync.dma_start(xc[c][:, rsl], xTr[c][:, rsl])

        padf = const.tile([128, B * NKT], f32)
        nc.vector.tensor_copy(padf, ids_sb)
        nc.vector.tensor_scalar_min(padf, padf, 1.0)

        # diagmask2[x, h, y] = 1 if y >= x else 0 (keys on partitions)
        diagmask = const.tile([128, 128], bf16)
        nc.gpsimd.memset(diagmask, 1.0)
        nc.gpsimd.affine_select(
            out=diagmask, in_=diagmask, compare_op=ALU.is_ge, fill=0.0,
            base=0, pattern=[[1, 128]], channel_multiplier=-1,
        )
        diagmask2 = const.tile([128, 2, 128], bf16)
        nc.vector.tensor_copy(diagmask2[:, 0, :], diagmask)
        nc.vector.tensor_copy(diagmask2[:, 1, :], diagmask)

        # ---- per-batch persistent tiles ----
        qt = {}
        kt_ = {}
        vaug = {}
        stage = {}
        ests = {}
        pos = {}
        recbs = {}
        a2a_outs = {}

        EST_BUFS = [2, 2, 1, 1]

        def get_batch_tiles(b):
            if b in qt:
                return
            qt[b] = qkp.tile([128, S], bf16, name=f"qt{b}", tag="qt")
            kt_[b] = qkp.tile([128, S], bf16, name=f"kt{b}", tag="kt")
            vaug[b] = qkp.tile([128, 2, NKT, HD + 1], bf16, name=f"vaug{b}",
                               tag="vaug")
            stage[b] = stg.tile([128, S], bf16, name=f"stage{b}", tag="stage")
            ests[b] = [estp.tile([128, 2, blocklen], bf16, name=f"est{c}",
                                 tag=f"est{c}", bufs=EST_BUFS[c])
                       for c, (_, blocklen) in enumerate(layout)]

        # ---- worklets ----
        def qk(b, rb):
            get_batch_tiles(b)
            rsl = slice(rb * 512, (rb + 1) * 512)
            pqt = ppool.tile([128, 512], f32, name="pqt", tag="pp")
            pkt = ppool.tile([128, 512], f32, name="pkt", tag="pp")
            for c in range(NCH):
                st = c == 0
                sp = c == NCH - 1
                rhs = xc[c][:, rsl]
                nc.tensor.matmul(pqt, wqT_sb[:, c, :], rhs, start=st, stop=sp)
                nc.tensor.matmul(pkt, wkT_sb[:, c, :], rhs, start=st, stop=sp)
            nc.vector.tensor_scalar_add(qt[b][:, rsl], pqt, bq_col)
            nc.vector.tensor_scalar_add(kt_[b][:, rsl], pkt, bk_col)

        def vproj(b, m):
            msl = slice(m * 128, (m + 1) * 128)
            pv = ppool.tile([128, LD], f32, name="pv", tag="pp")
            for c in range(NCH):
                nc.tensor.matmul(pv, xc[c][:, msl], wvT_sb[:, c, :],
                                 start=(c == 0), stop=(c == NCH - 1))
            tv = work.tile([128, LD], f32, name="tv", tag="tv")
            nc.vector.tensor_add(tv, pv, bv_bc)
            pcol = padf[:, b * NKT + m:b * NKT + m + 1]
            for h in range(2):
                nc.vector.tensor_scalar_mul(
                    vaug[b][:, h, m, 0:HD], tv[:, h * HD:(h + 1) * HD], pcol)
                nc.vector.tensor_copy(vaug[b][:, h, m, HD:HD + 1], pcol)

        def sc(b, c, e):
            kt, w, qs, off = layout[c][0][e]
            est = ests[b][c]
            ksl = slice(kt * 128, (kt + 1) * 128)
            ps = spool.tile([128, 2, 512], f32, name="ps", tag="sp")
            nc.tensor.matmul(ps[:, 0, 0:w], kt_[b][0:64, ksl],
                             qt[b][0:64, qs:qs + w], start=True, stop=True)
            nc.tensor.matmul(ps[:, 1, 0:w], kt_[b][64:128, ksl],
                             qt[b][64:128, qs:qs + w], start=True, stop=True)
            nc.scalar.activation(est[:, :, off:off + w], ps[:, :, 0:w],
                                 AF.Exp, scale=0.125)
            if kt >= 4 * c:  # diagonal tile: causal mask
                nc.vector.tensor_mul(est[:, :, off:off + 128],
                                     est[:, :, off:off + 128], diagmask2)

        def pv(b, c):
            entries, _ = layout[c]
            est = ests[b][c]
            for h in range(2):
                po = pvpool.tile([128, 512], f32, name=f"po{h}", tag="po")
                pos[(b, c, h)] = po
                last = 4 * c + 3
                for kt, w, qs, off in entries:
                    po_off = qs - 512 * c
                    nc.tensor.matmul(po[0:HD + 1, po_off:po_off + w],
                                     vaug[b][:, h, kt, :],
                                     est[:, h, off:off + w],
                                     start=(kt == 0), stop=(kt == last))
                # start the reciprocal chain immediately (DVE) so the
                # broadcast matmul issued later never stalls the PE
                den = recp.tile([1, 512], f32, name="den", tag=f"den{h}")
                nc.vector.tensor_copy(den, po[HD:HD + 1, :])
                rec = recp.tile([1, 512], f32, name="rec", tag=f"rec{h}")
                nc.vector.reciprocal_approx_fast(rec, den)
                recb = recp.tile([1, 512], bf16, name="recb",
                                 tag=f"recb{h}", bufs=2)
                nc.vector.tensor_copy(recb, rec)
                recbs[(b, c, h)] = recb

        def div(b, c):
            # softmax division: broadcast 1/denominator to 64 partitions
            # via a tiny PE matmul into po's upper half, then multiply
            for h in range(2):
                nc.tensor.matmul(pos[(b, c, h)][64:128, :], ones64,
                                 recbs[(b, c, h)], start=True, stop=True,
                                 skip_group_check=True)
            for h in range(2):
                po = pos[(b, c, h)]
                rbc = recp.tile([HD, 512], bf16, name="rbc", tag=f"rbc{h}")
                nc.vector.tensor_copy(rbc, po[64:128, :])
                nc.vector.tensor_mul(
                    stage[b][h * HD:(h + 1) * HD, 512 * c:512 * (c + 1)],
                    po[0:HD, :], rbc)

        def a2a(b, h2):
            q0, q1 = 1024 * h2, 1024 * (h2 + 1)
            nq = (q1 - q0) // NCORES
            a2a_in = dpool.tile([NCORES * 128, nq], bf16,
                                name=f"a2ai{b}{h2}", tag="a2ai")
            nc.gpsimd.dma_start(
                a2a_in.rearrange("(j p) r -> p j r", p=128),
                stage[b][:, q0:q1].rearrange("p (j r) -> p j r", j=NCORES))
            a2a_out = dpool.tile([NCORES * 128, nq], bf16,
                                 name=f"a2ao{b}{h2}", tag="a2ao")
            nc.gpsimd.collective_compute(
                "AllToAll", ALU.bypass,
                replica_groups=[list(range(NCORES))],
                ins=[a2a_in.opt()], outs=[a2a_out.opt()])
            a2a_outs[(b, h2)] = a2a_out

        def op(b, h2):
            a2a_out = a2a_outs[(b, h2)]
            nq = 128
            a2a_sb = stg.tile([128, NCORES, nq], bf16, name=f"a2as{b}{h2}",
                              tag="a2as", bufs=3)
            nc.sync.dma_start(
                a2a_sb, a2a_out.rearrange("(j p) r -> p j r", p=128))
            r0 = (2 * b + h2) * 128
            for n in range(D // 512):
                pout = ppool.tile([128, 512], f32, name="pout", tag="pp")
                for c in range(NCH):
                    nc.tensor.matmul(
                        pout, a2a_sb[:, c, :],
                        woT_sb[:, c, n * 512:(n + 1) * 512],
                        start=(c == 0), stop=(c == NCH - 1))
                ot = work.tile([128, 512], f32, name="ot", tag="ot")
                nc.vector.tensor_add(ot, pout, bo_bc[:, n * 512:(n + 1) * 512])
                nc.sync.dma_start(
                    out.ap()[r0:r0 + 128, n * 512:(n + 1) * 512], ot)

        def xc_load(b):
            for rb in range(4):
                rsl = slice(rb * 512, (rb + 1) * 512)
                dsl = slice(b * S + rb * 512, b * S + (rb + 1) * 512)
                for c in range(NCH):
                    nc.sync.dma_start(xc[c][:, rsl], xTr[c][:, dsl])

        # ---- schedule: fillers paced against the ACT (exp) queue ----
        # sc_ready: FIFO of score entries whose qt/kt row-blocks exist.
        sc_ready = []
        act_cost = [0.0]   # us of exp work emitted
        pe_cost = [0.0]    # us of filler PE work emitted

        def enqueue_scores(b, c):
            for e in range(len(layout[c][0])):
                sc_ready.append((b, c, e))

        def drain(us=None, upto=None):
            """Emit queued score entries: ~us worth of ACT time, or all
            entries up to and including stripe (b, c) if upto is set."""
            target = act_cost[0] + (us if us is not None else 0.0)
            while sc_ready:
                b, c, e = sc_ready[0]
                if upto is not None:
                    if (b, c) > upto:
                        break
                elif act_cost[0] >= target or act_cost[0] > pe_cost[0] + 6.0:
                    break
                sc_ready.pop(0)
                sc(b, c, e)
                w = layout[c][0][e][1]
                act_cost[0] += 2 * w * 0.00109 + 0.1

        def fill(fn, us, *args):
            fn(*args)
            pe_cost[0] += us
            drain(us)

        QK_US, V_US, OP_US = 3.8, 0.5, 3.8

        # batch 0 projections + scores
        for rb in range(4):
            fill(qk, QK_US, 0, rb)
            enqueue_scores(0, rb)
        for m in range(NKT):
            fill(vproj, V_US, 0, m)
        xc_load(1)

        # batch 1 projections woven with batch-0 PV/div; b0 A2As fire as
        # soon as each half is normalized.  b1's stripe-2/3 score entries
        # are enqueued only after div(1,1) so their causal-mask DVE ops
        # (which wait on late exps) sit behind the A2A-gating div
        # multiplies in the in-order DVE queue.
        fill(qk, QK_US, 1, 0)
        enqueue_scores(1, 0)
        drain(upto=(0, 0))
        fill(pv, 1.2, 0, 0)
        fill(qk, QK_US, 1, 1)
        enqueue_scores(1, 1)
        fill(div, 0.5, 0, 0)
        drain(upto=(0, 1))
        fill(pv, 3.0, 0, 1)
        fill(qk, QK_US, 1, 2)
        fill(div, 0.5, 0, 1)
        a2a(0, 0)
        drain(upto=(0, 2))
        fill(pv, 4.5, 0, 2)
        fill(qk, QK_US, 1, 3)
        fill(div, 0.5, 0, 2)
        drain(upto=(0, 3))
        fill(pv, 6.5, 0, 3)
        for m in range(NKT):
            fill(vproj, V_US, 1, m)
        fill(div, 0.5, 0, 3)
        a2a(0, 1)

        # batch 1 PV + A2As, output projections woven into exp-wait slack
        drain(upto=(1, 0))
        fill(pv, 1.2, 1, 0)
        fill(div, 0.5, 1, 0)
        drain(upto=(1, 1))
        fill(pv, 3.0, 1, 1)
        fill(div, 0.5, 1, 1)
        a2a(1, 0)
        enqueue_scores(1, 2)
        drain(3.0)
        fill(op, OP_US, 0, 0)
        drain(upto=(1, 2))
        fill(pv, 4.5, 1, 2)
        fill(div, 0.5, 1, 2)
        enqueue_scores(1, 3)
        drain(3.0)
        fill(op, OP_US, 0, 1)
        drain(6.0)
        fill(op, OP_US, 1, 0)
        drain(upto=(1, 3))
        fill(pv, 6.5, 1, 3)
        fill(div, 0.5, 1, 3)
        a2a(1, 1)
        fill(op, OP_US, 1, 1)

        assert not sc_ready

    nc.finalize()
    return nc


def _get_nc():
    with _lock:
        if "nc" not in _cache:
            _cache["nc"] = _build_nc()
        return _cache["nc"]


def _shard_inputs(x, input_ids, Wq, bq, Wk, bk, Wv, bv, Wo, bo):
    import ml_dtypes
    bf16 = ml_dtypes.bfloat16

    x = np.asarray(x, dtype=np.float32)
    xT = np.ascontiguousarray(x.reshape(R, D).T).astype(bf16)
    woT = np.ascontiguousarray(np.asarray(Wo, dtype=np.float32).T).astype(bf16)
    bo_f = np.asarray(bo, dtype=np.float32)
    ids = np.asarray(input_ids).astype(np.int32)
    # ids_r[p, b*NKT + t] = input_ids[b, t*128 + p]
    ids_r = np.ascontiguousarray(ids.reshape(B, NKT, 128).transpose(2, 0, 1)
                                 .reshape(128, B * NKT))
    Wq = np.asarray(Wq, dtype=np.float32)
    Wk = np.asarray(Wk, dtype=np.float32)
    Wv = np.asarray(Wv, dtype=np.float32)
    bq = np.asarray(bq, dtype=np.float32)
    bk = np.asarray(bk, dtype=np.float32)
    bv = np.asarray(bv, dtype=np.float32)

    in_maps = []
    for c in range(NCORES):
        sl = slice(c * LD, (c + 1) * LD)
        in_maps.append({
            "xT": xT,
            "wqT": np.ascontiguousarray(Wq[sl].T).astype(bf16),
            "wkT": np.ascontiguousarray(Wk[sl].T).astype(bf16),
            "wvT": np.ascontiguousarray(Wv[sl].T).astype(bf16),
            "woT": woT,
            "bq": bq[sl].copy(),
            "bk": bk[sl].copy(),
            "bv": bv[sl].copy(),
            "bo": bo_f,
            "ids": ids_r,
        })
    return in_maps


def run(trace=False, **inputs):
    """Run the kernel; returns (output, BassKernelResults)."""
    from concourse.bass_utils import run_bass_kernel_spmd

    nc = _get_nc()
    in_maps = _shard_inputs(**inputs)
    res = run_bass_kernel_spmd(nc, in_maps, core_ids=list(range(NCORES)),
                               trace=trace)
    full = np.empty((B, S, D), dtype=np.float32)
    for j in range(NCORES):
        o = np.asarray(res.results[j]["out"], dtype=np.float32)
        for b in range(B):
            for h2 in range(2):
                full[b, 1024 * h2 + 128 * j:1024 * h2 + 128 * (j + 1), :] = \
                    o[(2 * b + h2) * 128:(2 * b + h2 + 1) * 128, :]
    return full, res


def kernel(**inputs) -> np.ndarray:
    full, _ = run(trace=False, **inputs)
    return full
